# revision 1
# baseline (speedup 1.0000x reference)
"""Multi-head attention (B=2, S=4096, D=768, H=12, d_k=64) on 8 TRN2 cores.

Sharding: core c -> batch b = c//4, head group g = c%4 (heads 3g..3g+2).
Each core computes partial = sum_{h in group} softmax(QK^T/8) V @ Wo_h^T
over its batch; host sums the 4 partials per batch and adds bo.

Device kernel (identical SPMD program, per-core data):
  Phase A: QKV projections (fp32r matmuls), Q^T/K^T/V^T produced in
           [head_dim, seq] layout (bf16), V transposed to natural
           [seq, head_dim] layout with a ones column appended (row sums).
  Phase B: per (head, q-chunk of 512): S^T tiles [128k, 512q] via
           64-contraction matmuls (two concurrent row-tiles T0/T8),
           exp on ACT from 2-bank PSUM groups -> bf16, O^T accumulation
           with V|ones (row 64 = softmax sums), per-q normalization via
           reciprocal + gpsimd partition broadcast.
  Phase C: out[qtile] = sum_h O_h^T.T @ Wo_h^T (fp32r), DMA to DRAM.
"""

import numpy as np

import concourse.bass as bass
import concourse.mybir as mybir
import concourse.tile as tile
from concourse import bacc
from concourse.masks import make_identity

F32 = mybir.dt.float32
F32R = mybir.dt.float32r
BF16 = mybir.dt.bfloat16

N_CORES = 8
B, S, D = 2, 4096, 768
H, DK = 12, 64
HPC = 3            # heads per core
QC = 512           # q-chunk width (free dim of S^T matmuls)
NQC = S // QC      # 8
NKB = S // 128     # 32 k-blocks of 128
XCH = 512          # x streaming chunk (columns of x^T per DMA)
OT_DT = F32R       # dtype of O^T staging

# projection group packing: 5 groups of two 64-dim tensors (by (head, kind))
# kind: 0=Q, 1=K, 2=V
PROJ_GROUPS = [((0, 0), (0, 1)), ((0, 2), (1, 0)), ((1, 1), (1, 2)),
               ((2, 0), (2, 1)), ((2, 2), (2, 2))]


def build_program(debug=False, repeat=1, mode="v2_e2"):
    nc = bacc.Bacc("TRN2", debug=False, num_devices=N_CORES)

    xT_d = nc.dram_tensor("xT", [D, S], F32R, kind="ExternalInput").ap()
    if mode.startswith("v2"):
        wp_d = nc.dram_tensor("wp", [HPC, 2, 6, 128, 128], F32R,
                              kind="ExternalInput").ap()
        bp_d = nc.dram_tensor("bp", [128, HPC, 2], F32,
                              kind="ExternalInput").ap()
    else:
        wp_d = nc.dram_tensor("wp", [5, 6, 128, 128], F32R,
                              kind="ExternalInput").ap()
        bp_d = nc.dram_tensor("bp", [128, 5], F32, kind="ExternalInput").ap()
    wo_d = nc.dram_tensor("wo", [HPC, DK, D], F32R, kind="ExternalInput").ap()
    out_d = nc.dram_tensor("out", [S, D], F32, kind="ExternalOutput").ap()

    dbg = {}
    if debug:
        dbg["qt"] = nc.dram_tensor("d_qt", [128, S], BF16,
                                   kind="ExternalOutput").ap()
        dbg["kt"] = nc.dram_tensor("d_kt", [128, S], BF16,
                                   kind="ExternalOutput").ap()
        dbg["v"] = nc.dram_tensor("d_v", [128, NKB, DK + 1], BF16,
                                  kind="ExternalOutput").ap()
        dbg["es"] = nc.dram_tensor("d_es", [128, 2, QC], BF16,
                                   kind="ExternalOutput").ap()
        dbg["po"] = nc.dram_tensor("d_po", [2, DK + 1, QC], F32,
                                   kind="ExternalOutput").ap()
        dbg["otr"] = nc.dram_tensor("d_otr", [DK + 1, QC], F32,
                                    kind="ExternalOutput").ap()
        dbg["rbc"] = nc.dram_tensor("d_rbc", [DK + 1, QC], F32,
                                    kind="ExternalOutput").ap()
        dbg["ot"] = nc.dram_tensor("d_ot", [DK + 1, S], F32,
                                   kind="ExternalOutput").ap()

    with tile.TileContext(nc) as tc, \
            nc.allow_low_precision("bf16/fp32r attention pipeline"):
        if mode.startswith("v2"):
            assert not debug and repeat >= 1
            for _ in range(repeat):
                _emit_v2(nc, tc, xT_d, wp_d, bp_d, wo_d, out_d,
                         exp_group=4 if mode == "v2_e4" else 2)
        else:
            _emit(nc, tc, xT_d, wp_d, bp_d, wo_d, out_d, dbg,
                  repeat=repeat, mode=mode)
    nc.compile()
    return nc


def _emit(nc, tc, xT_d, wp_d, bp_d, wo_d, out_d, dbg={},
          repeat=1, mode="tiled64"):
    import contextlib
    ctx = contextlib.ExitStack()
    with ctx:
        wpool = ctx.enter_context(tc.tile_pool(name="wpool", bufs=1))
        persist = ctx.enter_context(tc.tile_pool(name="persist", bufs=1))
        xpool = ctx.enter_context(tc.tile_pool(name="xpool", bufs=2))
        epool = ctx.enter_context(tc.tile_pool(name="epool", bufs=3))
        rpool = ctx.enter_context(tc.tile_pool(name="rpool", bufs=1))
        opool = ctx.enter_context(tc.tile_pool(name="opool", bufs=2))
        ppS = ctx.enter_context(tc.tile_pool(name="ppS", bufs=2, space="PSUM"))
        ppO = ctx.enter_context(tc.tile_pool(name="ppO", bufs=1, space="PSUM"))
        ppA = ctx.enter_context(tc.tile_pool(name="ppA", bufs=2, space="PSUM"))

        # ---- constants / weights ----
        wsb = wpool.tile([128, 5, 6, 128], F32R)
        nc.sync.dma_start(out=wsb, in_=wp_d.rearrange("g c p m -> p g c m"))
        bsb = wpool.tile([128, 5], F32)
        nc.sync.dma_start(out=bsb, in_=bp_d)
        wosb = wpool.tile([DK, HPC, D], F32R)
        nc.sync.dma_start(out=wosb, in_=wo_d.rearrange("j d m -> d j m"))
        ident = wpool.tile([128, 128], BF16)
        make_identity(nc, ident)

        assert not (dbg and repeat > 1)
        # which half each (head, kind) tensor is written to by the packed
        # projections, derived from PROJ_GROUPS
        wr_half = {}
        for gi, (mA, mB) in enumerate(PROJ_GROUPS):
            if gi == 4:
                wr_half[mA] = 0  # written to both halves
                continue
            wr_half[mA] = 0
            wr_half[mB] = 1

        for rep in range(repeat):
            # ---- persistent per-head tensors ----
            # QT/KT: [head_dim(64) in both halves (tiled64) or lower half +
            # zero upper (pad128), seq] bf16
            QT = [persist.tile([128, S], BF16, tag=f"qt{j}", name=f"qt{j}")
                  for j in range(HPC)]
            KT = [persist.tile([128, S], BF16, tag=f"kt{j}", name=f"kt{j}")
                  for j in range(HPC)]
            # V natural layout + ones column: [128 part = k%128, kb, 65]
            V = [persist.tile([128, NKB, DK + 1], BF16, tag=f"v{j}",
                              name=f"v{j}") for j in range(HPC)]
            # O^T staging: rows 0..63 = head dims, row 64 = softmax sums
            OT = [persist.tile([DK + 1, S], OT_DT, tag=f"ot{j}",
                               name=f"ot{j}") for j in range(HPC)]
            # VT transient [dims(64) at written half, seq] bf16
            VT = [persist.tile([128, S], BF16, tag=f"vt{j}", name=f"vt{j}")
                  for j in range(HPC)]

            for j in range(HPC):
                nc.vector.memset(V[j][:, :, DK], 1.0)

            def tgt(j, kind):
                return QT[j] if kind == 0 else KT[j] if kind == 1 else VT[j]

            # ---- Phase A: projections, x streamed in contraction-complete
            # column chunks ----
            n_xch = S // XCH
            for ci in range(n_xch):
                xq = xpool.tile([128, 6, XCH], F32R, tag="x", name="xq")
                nc.sync.dma_start(
                    out=xq,
                    in_=xT_d.rearrange("(c p) q -> p c q", p=128)[
                        :, :, ci * XCH:(ci + 1) * XCH],
                )
                for gi, (mA, mB) in enumerate(PROJ_GROUPS):
                    ps = ppA.tile([128, XCH], F32, tag="s", name="ps")
                    for c in range(6):
                        nc.tensor.matmul(
                            ps, lhsT=wsb[:, gi, c, :], rhs=xq[:, c, :],
                            start=(c == 0), stop=(c == 5))
                    # evacuate halves with bias add, cast to bf16
                    if gi == 4:
                        # V2 written to both halves at once (dup'd weights)
                        nc.vector.tensor_scalar_add(
                            out=VT[2][:, ci * XCH:(ci + 1) * XCH],
                            in0=ps, scalar1=bsb[:, gi:gi + 1])
                        continue
                    for half, (j, kind) in ((0, mA), (1, mB)):
                        lo, hi = half * 64, half * 64 + 64
                        nc.vector.tensor_scalar_add(
                            out=tgt(j, kind)[lo:hi, ci * XCH:(ci + 1) * XCH],
                            in0=ps[lo:hi, :],
                            scalar1=bsb[lo:hi, gi:gi + 1])

            # fix up Q/K halves (V^T needs none: transposes read the
            # written half directly)
            for j in range(HPC):
                for kind in (0, 1):
                    t = tgt(j, kind)
                    wh = wr_half[(j, kind)]
                    lo, hi = wh * 64, wh * 64 + 64
                    olo, ohi = 64 - lo, 128 - lo
                    if mode == "tiled64":
                        # duplicate into the other half
                        nc.sync.dma_start(out=t[olo:ohi, :], in_=t[lo:hi, :])
                    else:
                        # data to lower half, zero upper
                        if wh == 1:
                            nc.sync.dma_start(out=t[0:64, :], in_=t[64:128, :])
                        nc.vector.memset(t[64:128, :], 0.0)

            # V: transpose VT [dims, seq] -> natural [seq, dims] per block
            for j in range(HPC):
                voff = wr_half[(j, 2)] * 64
                for kb in range(NKB):
                    pt = ppA.tile([128, 128], BF16, tag="s", name="pt")
                    nc.tensor.transpose(
                        pt, VT[j][:, kb * 128:(kb + 1) * 128], ident)
                    nc.vector.tensor_copy(
                        out=V[j][:, kb, 0:DK], in_=pt[:, voff:voff + DK])

            if dbg:
                nc.sync.dma_start(out=dbg["qt"], in_=QT[0])
                nc.sync.dma_start(out=dbg["kt"], in_=KT[0])
                nc.sync.dma_start(out=dbg["v"], in_=V[0])

            # ---- Phase B: attention per head ----
            for j in range(HPC):
                for qi in range(NQC):
                    qs = qi * QC
                    poa = ppO.tile([DK + 1, QC], F32, tag="oa", name="poa")
                    if mode == "tiled64":
                        pob = ppO.tile([DK + 1, QC], F32, tag="ob",
                                       name="pob")
                    for p in range(NKB // 2):  # pairs of k-blocks
                        pss = ppS.tile([128, 2, QC], F32, tag="s", name="pss")
                        if mode == "tiled64":
                            # two concurrent 64-contraction row tiles
                            nc.tensor.matmul(
                                pss[:, 0, :],
                                lhsT=KT[j][0:64, p * 256:p * 256 + 128],
                                rhs=QT[j][0:64, qs:qs + QC],
                                start=True, stop=True)
                            nc.tensor.matmul(
                                pss[:, 1, :],
                                lhsT=KT[j][64:128, p * 256 + 128:p * 256 + 256],
                                rhs=QT[j][64:128, qs:qs + QC],
                                start=True, stop=True)
                        else:
                            for s in range(2):
                                kb = 2 * p + s
                                nc.tensor.matmul(
                                    pss[:, s, :],
                                    lhsT=KT[j][:, kb * 128:(kb + 1) * 128],
                                    rhs=QT[j][:, qs:qs + QC],
                                    start=True, stop=True)
                        es = epool.tile([128, 2, QC], BF16, tag="e", name="es")
                        nc.scalar.activation(
                            out=es, in_=pss,
                            func=mybir.ActivationFunctionType.Exp, scale=0.125)
                        if dbg and j == 0 and qi == 0 and p == 0:
                            nc.sync.dma_start(out=dbg["es"], in_=es)
                        for s in range(2):
                            kb = 2 * p + s
                            first = p == 0 and s == 0
                            last = p == NKB // 2 - 1 and s == 1
                            if mode == "tiled64":
                                nc.tensor.matmul(
                                    poa, lhsT=V[j][0:64, kb, :],
                                    rhs=es[0:64, s, :],
                                    start=first, stop=last,
                                    skip_group_check=True)
                                nc.tensor.matmul(
                                    pob, lhsT=V[j][64:128, kb, :],
                                    rhs=es[64:128, s, :],
                                    start=first, stop=last,
                                    skip_group_check=True)
                            else:
                                nc.tensor.matmul(
                                    poa, lhsT=V[j][:, kb, :],
                                    rhs=es[:, s, :],
                                    start=first, stop=last,
                                    skip_group_check=True)
                    # evacuate (DVE may read only one PSUM operand per op)
                    nc.vector.tensor_copy(out=OT[j][:, qs:qs + QC], in_=poa)
                    if mode == "tiled64":
                        nc.vector.tensor_add(
                            out=OT[j][:, qs:qs + QC],
                            in0=OT[j][:, qs:qs + QC], in1=pob)
                    # reciprocal of sums in place (row 64)
                    nc.vector.reciprocal(
                        out=OT[j][DK:DK + 1, qs:qs + QC],
                        in_=OT[j][DK:DK + 1, qs:qs + QC])
                    if dbg and j == 0 and qi == 0:
                        nc.sync.dma_start(
                            out=dbg["otr"],
                            in_=OT[0][:, 0:QC].bitcast(F32))
                    # broadcast recip across partitions and scale O^T.
                    # partition_broadcast reads PHYSICAL partition 0, so
                    # stage the recip row there via a tiny DMA first.
                    srow = rpool.tile([1, QC], OT_DT, tag="sr", name="srow")
                    nc.sync.dma_start(
                        out=srow, in_=OT[j][DK:DK + 1, qs:qs + QC])
                    rbc = rpool.tile([DK + 1, QC], OT_DT, tag="r", name="rbc")
                    nc.gpsimd.partition_broadcast(rbc, srow, channels=DK + 1)
                    if dbg and j == 0 and qi == 0:
                        nc.sync.dma_start(out=dbg["rbc"], in_=rbc.bitcast(F32))
                    nc.vector.tensor_mul(
                        out=OT[j][0:DK, qs:qs + QC],
                        in0=OT[j][0:DK, qs:qs + QC], in1=rbc[0:DK, :])

            if dbg:
                nc.sync.dma_start(out=dbg["ot"], in_=OT[0].bitcast(F32))

            # ---- Phase C: output projection ----
            for t in range(S // 128):
                c1 = ppA.tile([128, 512], F32, tag="s", name="c1")
                c2 = ppA.tile([128, 256], F32, tag="s", name="c2")
                for j in range(HPC):
                    nc.tensor.matmul(
                        c1, lhsT=OT[j][0:DK, t * 128:(t + 1) * 128],
                        rhs=wosb[:, j, 0:512],
                        start=(j == 0), stop=(j == HPC - 1))
                for j in range(HPC):
                    nc.tensor.matmul(
                        c2, lhsT=OT[j][0:DK, t * 128:(t + 1) * 128],
                        rhs=wosb[:, j, 512:768],
                        start=(j == 0), stop=(j == HPC - 1))
                ot = opool.tile([128, D], F32, tag="o", name="ot")
                nc.vector.tensor_copy(out=ot[:, 0:512], in_=c1)
                nc.vector.tensor_copy(out=ot[:, 512:768], in_=c2)
                nc.sync.dma_start(out=out_d[t * 128:(t + 1) * 128, :], in_=ot)




def _emit_v2(nc, tc, xT_d, wp_d, bp_d, wo_d, out_d, exp_group=4):
    """Per-head pipeline; S^T psum in bf16 when exp_group=4 (2048-wide exp)."""
    import contextlib
    ctx = contextlib.ExitStack()
    with ctx:
        wpool = ctx.enter_context(tc.tile_pool(name="wpool", bufs=1))
        persist = ctx.enter_context(tc.tile_pool(name="persist", bufs=1))
        hpool = ctx.enter_context(tc.tile_pool(name="hpool", bufs=2))
        xpool = ctx.enter_context(tc.tile_pool(name="xpool", bufs=2))
        epool = ctx.enter_context(tc.tile_pool(name="epool", bufs=4))
        rpool = ctx.enter_context(tc.tile_pool(name="rpool", bufs=1))
        opool = ctx.enter_context(tc.tile_pool(name="opool", bufs=2))
        # one shared PSUM pool for S-groups/proj/transposes/phase C
        # (3 slots of 2 banks) + the two O accumulators (1 bank each)
        ppS = ctx.enter_context(tc.tile_pool(name="ppS", bufs=3, space="PSUM"))
        ppO = ctx.enter_context(tc.tile_pool(name="ppO", bufs=1, space="PSUM"))
        ppA = ppS

        SDT = BF16 if exp_group == 4 else F32
        NG = NKB // exp_group

        wsb = wpool.tile([128, HPC, 2, 6, 128], F32R)
        nc.sync.dma_start(out=wsb, in_=wp_d.rearrange("j g c p m -> p j g c m"))
        bsb = wpool.tile([128, HPC, 2], F32)
        nc.sync.dma_start(out=bsb, in_=bp_d)
        wosb = wpool.tile([DK, HPC, D], F32R)
        nc.sync.dma_start(out=wosb, in_=wo_d.rearrange("j d m -> d j m"))
        ident = wpool.tile([128, 128], BF16)
        make_identity(nc, ident)

        OT = [persist.tile([DK + 1, S], OT_DT, tag=f"ot{j}", name=f"ot{j}")
              for j in range(HPC)]

        def emit_c(cqi):
            for t in range(cqi * QC // 128, (cqi + 1) * QC // 128):
                c1 = ppO.tile([128, 512], F32, tag="oa", name="c1")
                c2 = ppO.tile([128, 256], F32, tag="ob", name="c2")
                for jj in range(HPC):
                    nc.tensor.matmul(
                        c1, lhsT=OT[jj][0:DK, t * 128:(t + 1) * 128],
                        rhs=wosb[:, jj, 0:512],
                        start=(jj == 0), stop=(jj == HPC - 1))
                for jj in range(HPC):
                    nc.tensor.matmul(
                        c2, lhsT=OT[jj][0:DK, t * 128:(t + 1) * 128],
                        rhs=wosb[:, jj, 512:768],
                        start=(jj == 0), stop=(jj == HPC - 1))
                ot = opool.tile([128, D], F32, tag="o", name="ot")
                nc.vector.tensor_copy(out=ot[:, 0:512], in_=c1)
                nc.vector.tensor_copy(out=ot[:, 512:768], in_=c2)
                nc.sync.dma_start(
                    out=out_d[t * 128:(t + 1) * 128, :], in_=ot)

        n_xch = S // XCH
        for j in range(HPC):
            # ---- phase A for head j ----
            QT = hpool.tile([128, S], BF16, tag="qt", name="qt")
            KT = hpool.tile([128, S], BF16, tag="kt", name="kt")
            VT = hpool.tile([128, S], BF16, tag="vt", name="vt")
            V = hpool.tile([128, NKB, DK + 1], BF16, tag="v", name="v")
            nc.vector.memset(V[:, :, DK], 1.0)
            for ci in range(n_xch):
                xq = xpool.tile([128, 6, XCH], F32R, tag="x", name="xq")
                nc.sync.dma_start(
                    out=xq,
                    in_=xT_d.rearrange("(c p) q -> p c q", p=128)[
                        :, :, ci * XCH:(ci + 1) * XCH])
                cs = slice(ci * XCH, (ci + 1) * XCH)
                # group 0: (Q | K)
                ps = ppA.tile([128, XCH], F32, tag="s", name="ps")
                for c in range(6):
                    nc.tensor.matmul(
                        ps, lhsT=wsb[:, j, 0, c, :], rhs=xq[:, c, :],
                        start=(c == 0), stop=(c == 5))
                nc.vector.tensor_scalar_add(
                    out=QT[0:64, cs], in0=ps[0:64, :],
                    scalar1=bsb[0:64, j, 0:1])
                nc.vector.tensor_scalar_add(
                    out=KT[64:128, cs], in0=ps[64:128, :],
                    scalar1=bsb[64:128, j, 0:1])
                # group 1: (V | V) duplicated
                ps2 = ppA.tile([128, XCH], F32, tag="s", name="ps2")
                for c in range(6):
                    nc.tensor.matmul(
                        ps2, lhsT=wsb[:, j, 1, c, :], rhs=xq[:, c, :],
                        start=(c == 0), stop=(c == 5))
                nc.vector.tensor_scalar_add(
                    out=VT[:, cs], in0=ps2, scalar1=bsb[:, j, 1:2])
                # V natural layout via PE transposes (chunk's k-blocks)
                for kb in range(ci * XCH // 128, (ci + 1) * XCH // 128):
                    pt = ppA.tile([128, 128], BF16, tag="s", name="pt")
                    nc.tensor.transpose(
                        pt, VT[:, kb * 128:(kb + 1) * 128], ident)
                    nc.vector.tensor_copy(
                        out=V[:, kb, 0:DK], in_=pt[:, 0:DK])
            # duplicate halves: Q lower->upper, K upper->lower
            nc.sync.dma_start(out=QT[64:128, :], in_=QT[0:64, :])
            nc.sync.dma_start(out=KT[0:64, :], in_=KT[64:128, :])

            # ---- phase B for head j ----
            for qi in range(NQC):
                qs = qi * QC
                poa = ppO.tile([DK + 1, QC], F32, tag="oa", name="poa")
                pob = ppO.tile([DK + 1, QC], F32, tag="ob", name="pob")
                for g in range(NG):
                    pss = ppS.tile([128, exp_group, QC], SDT, tag="s",
                                   name="pss")
                    # T0 row-tile: first half of the group's k-blocks;
                    # T8: second half (separate PSUM banks)
                    hg = exp_group // 2
                    for i in range(hg):
                        kb = g * exp_group + i
                        nc.tensor.matmul(
                            pss[:, i, :],
                            lhsT=KT[0:64, kb * 128:(kb + 1) * 128],
                            rhs=QT[0:64, qs:qs + QC],
                            start=True, stop=True)
                    for i in range(hg):
                        kb = g * exp_group + hg + i
                        nc.tensor.matmul(
                            pss[:, hg + i, :],
                            lhsT=KT[64:128, kb * 128:(kb + 1) * 128],
                            rhs=QT[64:128, qs:qs + QC],
                            start=True, stop=True)
                    es = epool.tile([128, exp_group, QC], BF16, tag="e",
                                    name="es")
                    nc.scalar.activation(
                        out=es, in_=pss,
                        func=mybir.ActivationFunctionType.Exp, scale=0.125)
                    for s in range(exp_group):
                        kb = g * exp_group + s
                        first = g == 0 and s == 0
                        last = g == NG - 1 and s == exp_group - 1
                        nc.tensor.matmul(
                            poa, lhsT=V[0:64, kb, :], rhs=es[0:64, s, :],
                            start=first, stop=last, skip_group_check=True)
                        nc.tensor.matmul(
                            pob, lhsT=V[64:128, kb, :], rhs=es[64:128, s, :],
                            start=first, stop=last, skip_group_check=True)
                nc.vector.tensor_copy(out=OT[j][:, qs:qs + QC], in_=poa)
                nc.vector.tensor_add(
                    out=OT[j][:, qs:qs + QC],
                    in0=OT[j][:, qs:qs + QC], in1=pob)
                nc.vector.reciprocal(
                    out=OT[j][DK:DK + 1, qs:qs + QC],
                    in_=OT[j][DK:DK + 1, qs:qs + QC])
                srow = rpool.tile([1, QC], OT_DT, tag="sr", name="srow")
                nc.sync.dma_start(
                    out=srow, in_=OT[j][DK:DK + 1, qs:qs + QC])
                rbc = rpool.tile([DK + 1, QC], OT_DT, tag="r", name="rbc")
                nc.gpsimd.partition_broadcast(rbc, srow, channels=DK + 1)
                nc.vector.tensor_mul(
                    out=OT[j][0:DK, qs:qs + QC],
                    in0=OT[j][0:DK, qs:qs + QC], in1=rbc[0:DK, :])

        # ---- phase C: output projection (borrows psumO slots) ----
        for cqi in range(NQC):
            emit_c(cqi)




# ---------------------------------------------------------------------------
# host side
# ---------------------------------------------------------------------------

KERNEL_MODE = "v2_e2"


def shard_inputs(x, Wq, bq, Wk, bk, Wv, bv, Wo, bo, mode=None):
    """Build the 8 per-core input maps."""
    mode = mode or KERNEL_MODE
    if mode.startswith("v2"):
        return shard_inputs_v2(x, Wq, bq, Wk, bk, Wv, bv, Wo, bo)
    return shard_inputs_v1(x, Wq, bq, Wk, bk, Wv, bv, Wo, bo)


def shard_inputs_v2(x, Wq, bq, Wk, bk, Wv, bv, Wo, bo):
    x = np.asarray(x, np.float32)
    Wq, Wk, Wv = (np.asarray(a, np.float32) for a in (Wq, Wk, Wv))
    bq, bk, bv = (np.asarray(a, np.float32) for a in (bq, bk, bv))
    Wo = np.asarray(Wo, np.float32)
    in_maps = []
    for c in range(N_CORES):
        b, g = divmod(c, 4)
        heads = [3 * g + j for j in range(HPC)]
        wp = np.empty((HPC, 2, 6, 128, 128), np.float32)
        bp = np.zeros((128, HPC, 2), np.float32)
        wo = np.empty((HPC, DK, D), np.float32)
        for j, h in enumerate(heads):
            sl = slice(64 * h, 64 * h + 64)
            wp[j, 0, :, :, 0:64] = Wq[sl].T.reshape(6, 128, 64)
            wp[j, 0, :, :, 64:128] = Wk[sl].T.reshape(6, 128, 64)
            wp[j, 1, :, :, 0:64] = Wv[sl].T.reshape(6, 128, 64)
            wp[j, 1, :, :, 64:128] = Wv[sl].T.reshape(6, 128, 64)
            bp[0:64, j, 0] = bq[sl]
            bp[64:128, j, 0] = bk[sl]
            bp[0:64, j, 1] = bv[sl]
            bp[64:128, j, 1] = bv[sl]
            wo[j] = Wo[:, sl].T
        in_maps.append({
            "xT": np.ascontiguousarray(x[b].T),
            "wp": wp, "bp": bp, "wo": wo,
        })
    return in_maps


def shard_inputs_v1(x, Wq, bq, Wk, bk, Wv, bv, Wo, bo):
    """Build the 8 per-core input maps."""
    x = np.asarray(x, np.float32)
    Ws = {0: np.asarray(Wq, np.float32), 1: np.asarray(Wk, np.float32),
          2: np.asarray(Wv, np.float32)}
    bs = {0: np.asarray(bq, np.float32), 1: np.asarray(bk, np.float32),
          2: np.asarray(bv, np.float32)}
    Wo = np.asarray(Wo, np.float32)
    in_maps = []
    for c in range(N_CORES):
        b, g = divmod(c, 4)
        heads = [3 * g + j for j in range(HPC)]
        wp = np.empty((5, 6, 128, 128), np.float32)
        bp = np.zeros((128, 5), np.float32)
        for gi, (mA, mB) in enumerate(PROJ_GROUPS):
            for half, (j, kind) in ((0, mA), (1, mB)):
                h = heads[j]
                Wh = Ws[kind][64 * h:64 * h + 64, :]       # [64, 768]
                chunks = Wh.T.reshape(6, 128, 64)          # [c, p, 64]
                wp[gi, :, :, half * 64:half * 64 + 64] = chunks
                bp[half * 64:half * 64 + 64, gi] = bs[kind][64 * h:64 * h + 64]
        wo = np.empty((HPC, DK, D), np.float32)
        for j in range(HPC):
            h = heads[j]
            wo[j] = Wo[:, 64 * h:64 * h + 64].T
        in_maps.append({
            "xT": np.ascontiguousarray(x[b].T),
            "wp": wp, "bp": bp, "wo": wo,
        })
    return in_maps


def assemble_output(parts, bo):
    out = np.empty((B, S, D), np.float32)
    for b in range(B):
        acc = parts[4 * b]["out"].astype(np.float32).copy()
        for c in range(4 * b + 1, 4 * b + 4):
            acc += parts[c]["out"]
        out[b] = acc + np.asarray(bo, np.float32)[None, :]
    return out


_RUNNER = None


def _make_runner(nc):
    """Reusable PJRT runner (mirrors bass2jax.run_bass_via_pjrt multi-core)."""
    import jax
    import jax.numpy as jnp
    from jax.experimental.shard_map import shard_map
    from jax.sharding import Mesh, PartitionSpec
    from concourse import bass2jax

    bass2jax.install_neuronx_cc_hook()

    partition_name = (nc.partition_id_tensor.name
                      if nc.partition_id_tensor else None)
    in_names, out_names, out_avals = [], [], []
    for alloc in nc.m.functions[0].allocations:
        if not isinstance(alloc, mybir.MemoryLocationSet):
            continue
        name = alloc.memorylocations[0].name
        if alloc.kind == "ExternalInput":
            if name != partition_name:
                in_names.append(name)
        elif alloc.kind == "ExternalOutput":
            out_names.append(name)
            out_avals.append(jax.core.ShapedArray(
                tuple(alloc.tensor_shape), mybir.dt.np(alloc.dtype)))
    n_params = len(in_names)
    n_outs = len(out_names)
    all_in_names = list(in_names) + list(out_names)
    if partition_name is not None:
        all_in_names.append(partition_name)
    donate = tuple(range(n_params, n_params + n_outs))

    def _body(*args):
        operands = list(args)
        if partition_name is not None:
            operands.append(bass2jax.partition_id_tensor())
        outs = bass2jax._bass_exec_p.bind(
            *operands,
            out_avals=tuple(out_avals),
            in_names=tuple(all_in_names),
            out_names=tuple(out_names),
            lowering_input_output_aliases=(),
            sim_require_finite=True,
            sim_require_nnan=True,
            nc=nc,
        )
        return tuple(outs)

    devices = jax.devices()[:N_CORES]
    mesh = Mesh(np.asarray(devices), ("core",))
    in_specs = (PartitionSpec("core"),) * (n_params + n_outs)
    out_specs = (PartitionSpec("core"),) * n_outs
    sharded = jax.jit(
        shard_map(_body, mesh=mesh, in_specs=in_specs, out_specs=out_specs,
                  check_rep=False),
        donate_argnums=donate, keep_unused=True)

    def run(in_maps):
        per_core = [[np.asarray(m[name]) for name in in_names]
                    for m in in_maps]
        concat_in = [np.concatenate([per_core[c][i] for c in range(N_CORES)],
                                    axis=0) for i in range(n_params)]
        zeros = [np.zeros((N_CORES * av.shape[0], *av.shape[1:]), av.dtype)
                 for av in out_avals]
        outs = sharded(*concat_in, *zeros)
        return [
            {name: np.asarray(outs[i]).reshape(N_CORES, *out_avals[i].shape)[c]
             for i, name in enumerate(out_names)}
            for c in range(N_CORES)
        ]

    run.sharded = sharded
    run.in_names = in_names
    run.out_names = out_names
    run.out_avals = out_avals
    run.n_params = n_params
    return run


def get_runner():
    global _RUNNER
    if _RUNNER is None:
        nc = build_program()
        _RUNNER = _make_runner(nc)
    return _RUNNER


def kernel(x, Wq, bq, Wk, bk, Wv, bv, Wo, bo):
    run = get_runner()
    in_maps = shard_inputs(x, Wq, bq, Wk, bk, Wv, bv, Wo, bo)
    parts = run(in_maps)
    return assemble_output(parts, bo)



# revision 31
# speedup vs baseline: 2.9657x; 2.9657x over previous
"""Multi-head attention (B=2, S=4096, D=768, H=12, d_k=64) on 8 TRN2 cores.

Sharding: core c -> batch b = c//4, head group g = c%4 (heads 3g..3g+2).
Each core computes partial = sum_{h in group} softmax(QK^T/8) V @ Wo_h^T
over its batch; host sums the 4 partials per batch and adds bo.

Device kernel (identical SPMD program, per-core data):
  Phase A: QKV projections (fp32r matmuls), Q^T/K^T/V^T produced in
           [head_dim, seq] layout (bf16), V transposed to natural
           [seq, head_dim] layout with a ones column appended (row sums).
  Phase B: per (head, q-chunk of 512): S^T tiles [128k, 512q] via
           64-contraction matmuls (two concurrent row-tiles T0/T8),
           exp on ACT from 2-bank PSUM groups -> bf16, O^T accumulation
           with V|ones (row 64 = softmax sums), per-q normalization via
           reciprocal + gpsimd partition broadcast.
  Phase C: out[qtile] = sum_h O_h^T.T @ Wo_h^T (fp32r), DMA to DRAM.
"""

import numpy as np

import concourse.bass as bass
import concourse.mybir as mybir
import concourse.tile as tile
from concourse import bacc
from concourse.masks import make_identity

F32 = mybir.dt.float32
F32R = mybir.dt.float32r
BF16 = mybir.dt.bfloat16
FP8 = mybir.dt.float8e4

N_CORES = 8
B, S, D = 2, 4096, 768
H, DK = 12, 64
HPC = 3            # heads per core
QC = 512           # q-chunk width (free dim of S^T matmuls)
NQC = S // QC      # 8
NKB = S // 128     # 32 k-blocks of 128
XCH = 512          # x streaming chunk (columns of x^T per DMA)
OT_DT = F32R       # dtype of O^T staging

# projection group packing: 5 groups of two 64-dim tensors (by (head, kind))
# kind: 0=Q, 1=K, 2=V
PROJ_GROUPS = [((0, 0), (0, 1)), ((0, 2), (1, 0)), ((1, 1), (1, 2)),
               ((2, 0), (2, 1)), ((2, 2), (2, 2))]


def build_program(debug=False, repeat=1, mode="v3"):
    nc = bacc.Bacc("TRN2", debug=False, num_devices=N_CORES)

    if mode.startswith("v3"):
        xT_d = nc.dram_tensor("xT", [D, S], BF16, kind="ExternalInput").ap()
        wqk_d = nc.dram_tensor("wqk", [HPC, 6, 128, 128], BF16,
                               kind="ExternalInput").ap()
        wv_d = nc.dram_tensor("wv", [HPC, 6, 128, DK], BF16,
                              kind="ExternalInput").ap()
        bqk_d = nc.dram_tensor("bqk", [128, HPC], F32,
                               kind="ExternalInput").ap()
        bv_d = nc.dram_tensor("bv", [1, HPC, DK], BF16,
                              kind="ExternalInput").ap()
        wo_d = nc.dram_tensor("wo", [HPC, DK, D], BF16,
                              kind="ExternalInput").ap()
        out_d = nc.dram_tensor("out", [S, D], F32, kind="ExternalOutput").ap()
        dbg = {}
        if mode == "v3dbg":
            dbg["qt"] = nc.dram_tensor("d_qt", [128, S], BF16,
                                       kind="ExternalOutput").ap()
            dbg["kt"] = nc.dram_tensor("d_kt", [128, S], BF16,
                                       kind="ExternalOutput").ap()
            dbg["v"] = nc.dram_tensor("d_v", [128, NKB // 2, 2, DK + 1], BF16,
                                      kind="ExternalOutput").ap()
            dbg["es"] = nc.dram_tensor("d_es", [128, 2, QC], BF16,
                                       kind="ExternalOutput").ap()
            dbg["otp"] = nc.dram_tensor("d_otp", [128, S], BF16,
                                        kind="ExternalOutput").ap()
            dbg["ot2"] = nc.dram_tensor("d_ot2", [DK, S], BF16,
                                        kind="ExternalOutput").ap()
            dbg["rbc"] = nc.dram_tensor("d_rbc", [DK, QC], F32,
                                        kind="ExternalOutput").ap()
        with tile.TileContext(nc) as tc, \
                nc.allow_low_precision("bf16 attention pipeline"):
            for _ in range(repeat):
                _emit_v3(nc, tc, xT_d, wqk_d, wv_d, bqk_d, bv_d, wo_d, out_d,
                         dbg=dbg)
        nc.compile()
        return nc

    xT_d = nc.dram_tensor("xT", [D, S], F32R, kind="ExternalInput").ap()
    if mode.startswith("v2"):
        wp_d = nc.dram_tensor("wp", [HPC, 2, 6, 128, 128], F32R,
                              kind="ExternalInput").ap()
        bp_d = nc.dram_tensor("bp", [128, HPC, 2], F32,
                              kind="ExternalInput").ap()
    else:
        wp_d = nc.dram_tensor("wp", [5, 6, 128, 128], F32R,
                              kind="ExternalInput").ap()
        bp_d = nc.dram_tensor("bp", [128, 5], F32, kind="ExternalInput").ap()
    wo_d = nc.dram_tensor("wo", [HPC, DK, D], F32R, kind="ExternalInput").ap()
    out_d = nc.dram_tensor("out", [S, D], F32, kind="ExternalOutput").ap()

    dbg = {}
    if debug:
        dbg["qt"] = nc.dram_tensor("d_qt", [128, S], BF16,
                                   kind="ExternalOutput").ap()
        dbg["kt"] = nc.dram_tensor("d_kt", [128, S], BF16,
                                   kind="ExternalOutput").ap()
        dbg["v"] = nc.dram_tensor("d_v", [128, NKB, DK + 1], BF16,
                                  kind="ExternalOutput").ap()
        dbg["es"] = nc.dram_tensor("d_es", [128, 2, QC], BF16,
                                   kind="ExternalOutput").ap()
        dbg["po"] = nc.dram_tensor("d_po", [2, DK + 1, QC], F32,
                                   kind="ExternalOutput").ap()
        dbg["otr"] = nc.dram_tensor("d_otr", [DK + 1, QC], F32,
                                    kind="ExternalOutput").ap()
        dbg["rbc"] = nc.dram_tensor("d_rbc", [DK + 1, QC], F32,
                                    kind="ExternalOutput").ap()
        dbg["ot"] = nc.dram_tensor("d_ot", [DK + 1, S], F32,
                                   kind="ExternalOutput").ap()

    with tile.TileContext(nc) as tc, \
            nc.allow_low_precision("bf16/fp32r attention pipeline"):
        if mode.startswith("v2"):
            assert not debug and repeat >= 1
            for _ in range(repeat):
                _emit_v2(nc, tc, xT_d, wp_d, bp_d, wo_d, out_d,
                         exp_group=4 if mode == "v2_e4" else 2)
        else:
            _emit(nc, tc, xT_d, wp_d, bp_d, wo_d, out_d, dbg,
                  repeat=repeat, mode=mode)
    nc.compile()
    return nc


def _emit(nc, tc, xT_d, wp_d, bp_d, wo_d, out_d, dbg={},
          repeat=1, mode="tiled64"):
    import contextlib
    ctx = contextlib.ExitStack()
    with ctx:
        wpool = ctx.enter_context(tc.tile_pool(name="wpool", bufs=1))
        persist = ctx.enter_context(tc.tile_pool(name="persist", bufs=1))
        xpool = ctx.enter_context(tc.tile_pool(name="xpool", bufs=2))
        epool = ctx.enter_context(tc.tile_pool(name="epool", bufs=3))
        rpool = ctx.enter_context(tc.tile_pool(name="rpool", bufs=1))
        opool = ctx.enter_context(tc.tile_pool(name="opool", bufs=2))
        ppS = ctx.enter_context(tc.tile_pool(name="ppS", bufs=2, space="PSUM"))
        ppO = ctx.enter_context(tc.tile_pool(name="ppO", bufs=1, space="PSUM"))
        ppA = ctx.enter_context(tc.tile_pool(name="ppA", bufs=2, space="PSUM"))

        # ---- constants / weights ----
        wsb = wpool.tile([128, 5, 6, 128], F32R)
        nc.sync.dma_start(out=wsb, in_=wp_d.rearrange("g c p m -> p g c m"))
        bsb = wpool.tile([128, 5], F32)
        nc.sync.dma_start(out=bsb, in_=bp_d)
        wosb = wpool.tile([DK, HPC, D], F32R)
        nc.sync.dma_start(out=wosb, in_=wo_d.rearrange("j d m -> d j m"))
        ident = wpool.tile([128, 128], BF16)
        make_identity(nc, ident)

        assert not (dbg and repeat > 1)
        # which half each (head, kind) tensor is written to by the packed
        # projections, derived from PROJ_GROUPS
        wr_half = {}
        for gi, (mA, mB) in enumerate(PROJ_GROUPS):
            if gi == 4:
                wr_half[mA] = 0  # written to both halves
                continue
            wr_half[mA] = 0
            wr_half[mB] = 1

        for rep in range(repeat):
            # ---- persistent per-head tensors ----
            # QT/KT: [head_dim(64) in both halves (tiled64) or lower half +
            # zero upper (pad128), seq] bf16
            QT = [persist.tile([128, S], BF16, tag=f"qt{j}", name=f"qt{j}")
                  for j in range(HPC)]
            KT = [persist.tile([128, S], BF16, tag=f"kt{j}", name=f"kt{j}")
                  for j in range(HPC)]
            # V natural layout + ones column: [128 part = k%128, kb, 65]
            V = [persist.tile([128, NKB, DK + 1], BF16, tag=f"v{j}",
                              name=f"v{j}") for j in range(HPC)]
            # O^T staging: rows 0..63 = head dims, row 64 = softmax sums
            OT = [persist.tile([DK + 1, S], OT_DT, tag=f"ot{j}",
                               name=f"ot{j}") for j in range(HPC)]
            # VT transient [dims(64) at written half, seq] bf16
            VT = [persist.tile([128, S], BF16, tag=f"vt{j}", name=f"vt{j}")
                  for j in range(HPC)]

            for j in range(HPC):
                nc.vector.memset(V[j][:, :, DK], 1.0)

            def tgt(j, kind):
                return QT[j] if kind == 0 else KT[j] if kind == 1 else VT[j]

            # ---- Phase A: projections, x streamed in contraction-complete
            # column chunks ----
            n_xch = S // XCH
            for ci in range(n_xch):
                xq = xpool.tile([128, 6, XCH], F32R, tag="x", name="xq")
                nc.sync.dma_start(
                    out=xq,
                    in_=xT_d.rearrange("(c p) q -> p c q", p=128)[
                        :, :, ci * XCH:(ci + 1) * XCH],
                )
                for gi, (mA, mB) in enumerate(PROJ_GROUPS):
                    ps = ppA.tile([128, XCH], F32, tag="s", name="ps")
                    for c in range(6):
                        nc.tensor.matmul(
                            ps, lhsT=wsb[:, gi, c, :], rhs=xq[:, c, :],
                            start=(c == 0), stop=(c == 5))
                    # evacuate halves with bias add, cast to bf16
                    if gi == 4:
                        # V2 written to both halves at once (dup'd weights)
                        nc.vector.tensor_scalar_add(
                            out=VT[2][:, ci * XCH:(ci + 1) * XCH],
                            in0=ps, scalar1=bsb[:, gi:gi + 1])
                        continue
                    for half, (j, kind) in ((0, mA), (1, mB)):
                        lo, hi = half * 64, half * 64 + 64
                        nc.vector.tensor_scalar_add(
                            out=tgt(j, kind)[lo:hi, ci * XCH:(ci + 1) * XCH],
                            in0=ps[lo:hi, :],
                            scalar1=bsb[lo:hi, gi:gi + 1])

            # fix up Q/K halves (V^T needs none: transposes read the
            # written half directly)
            for j in range(HPC):
                for kind in (0, 1):
                    t = tgt(j, kind)
                    wh = wr_half[(j, kind)]
                    lo, hi = wh * 64, wh * 64 + 64
                    olo, ohi = 64 - lo, 128 - lo
                    if mode == "tiled64":
                        # duplicate into the other half
                        nc.sync.dma_start(out=t[olo:ohi, :], in_=t[lo:hi, :])
                    else:
                        # data to lower half, zero upper
                        if wh == 1:
                            nc.sync.dma_start(out=t[0:64, :], in_=t[64:128, :])
                        nc.vector.memset(t[64:128, :], 0.0)

            # V: transpose VT [dims, seq] -> natural [seq, dims] per block
            for j in range(HPC):
                voff = wr_half[(j, 2)] * 64
                for kb in range(NKB):
                    pt = ppA.tile([128, 128], BF16, tag="s", name="pt")
                    nc.tensor.transpose(
                        pt, VT[j][:, kb * 128:(kb + 1) * 128], ident)
                    nc.vector.tensor_copy(
                        out=V[j][:, kb, 0:DK], in_=pt[:, voff:voff + DK])

            if dbg:
                nc.sync.dma_start(out=dbg["qt"], in_=QT[0])
                nc.sync.dma_start(out=dbg["kt"], in_=KT[0])
                nc.sync.dma_start(out=dbg["v"], in_=V[0])

            # ---- Phase B: attention per head ----
            for j in range(HPC):
                for qi in range(NQC):
                    qs = qi * QC
                    poa = ppO.tile([DK + 1, QC], F32, tag="oa", name="poa")
                    if mode == "tiled64":
                        pob = ppO.tile([DK + 1, QC], F32, tag="ob",
                                       name="pob")
                    for p in range(NKB // 2):  # pairs of k-blocks
                        pss = ppS.tile([128, 2, QC], F32, tag="s", name="pss")
                        if mode == "tiled64":
                            # two concurrent 64-contraction row tiles
                            nc.tensor.matmul(
                                pss[:, 0, :],
                                lhsT=KT[j][0:64, p * 256:p * 256 + 128],
                                rhs=QT[j][0:64, qs:qs + QC],
                                start=True, stop=True)
                            nc.tensor.matmul(
                                pss[:, 1, :],
                                lhsT=KT[j][64:128, p * 256 + 128:p * 256 + 256],
                                rhs=QT[j][64:128, qs:qs + QC],
                                start=True, stop=True)
                        else:
                            for s in range(2):
                                kb = 2 * p + s
                                nc.tensor.matmul(
                                    pss[:, s, :],
                                    lhsT=KT[j][:, kb * 128:(kb + 1) * 128],
                                    rhs=QT[j][:, qs:qs + QC],
                                    start=True, stop=True)
                        es = epool.tile([128, 2, QC], BF16, tag="e", name="es")
                        nc.scalar.activation(
                            out=es, in_=pss,
                            func=mybir.ActivationFunctionType.Exp, scale=0.125)
                        if dbg and j == 0 and qi == 0 and p == 0:
                            nc.sync.dma_start(out=dbg["es"], in_=es)
                        for s in range(2):
                            kb = 2 * p + s
                            first = p == 0 and s == 0
                            last = p == NKB // 2 - 1 and s == 1
                            if mode == "tiled64":
                                nc.tensor.matmul(
                                    poa, lhsT=V[j][0:64, kb, :],
                                    rhs=es[0:64, s, :],
                                    start=first, stop=last,
                                    skip_group_check=True)
                                nc.tensor.matmul(
                                    pob, lhsT=V[j][64:128, kb, :],
                                    rhs=es[64:128, s, :],
                                    start=first, stop=last,
                                    skip_group_check=True)
                            else:
                                nc.tensor.matmul(
                                    poa, lhsT=V[j][:, kb, :],
                                    rhs=es[:, s, :],
                                    start=first, stop=last,
                                    skip_group_check=True)
                    # evacuate (DVE may read only one PSUM operand per op)
                    nc.vector.tensor_copy(out=OT[j][:, qs:qs + QC], in_=poa)
                    if mode == "tiled64":
                        nc.vector.tensor_add(
                            out=OT[j][:, qs:qs + QC],
                            in0=OT[j][:, qs:qs + QC], in1=pob)
                    # reciprocal of sums in place (row 64)
                    nc.vector.reciprocal(
                        out=OT[j][DK:DK + 1, qs:qs + QC],
                        in_=OT[j][DK:DK + 1, qs:qs + QC])
                    if dbg and j == 0 and qi == 0:
                        nc.sync.dma_start(
                            out=dbg["otr"],
                            in_=OT[0][:, 0:QC].bitcast(F32))
                    # broadcast recip across partitions and scale O^T.
                    # partition_broadcast reads PHYSICAL partition 0, so
                    # stage the recip row there via a tiny DMA first.
                    srow = rpool.tile([1, QC], OT_DT, tag="sr", name="srow")
                    nc.sync.dma_start(
                        out=srow, in_=OT[j][DK:DK + 1, qs:qs + QC])
                    rbc = rpool.tile([DK + 1, QC], OT_DT, tag="r", name="rbc")
                    nc.gpsimd.partition_broadcast(rbc, srow, channels=DK + 1)
                    if dbg and j == 0 and qi == 0:
                        nc.sync.dma_start(out=dbg["rbc"], in_=rbc.bitcast(F32))
                    nc.vector.tensor_mul(
                        out=OT[j][0:DK, qs:qs + QC],
                        in0=OT[j][0:DK, qs:qs + QC], in1=rbc[0:DK, :])

            if dbg:
                nc.sync.dma_start(out=dbg["ot"], in_=OT[0].bitcast(F32))

            # ---- Phase C: output projection ----
            for t in range(S // 128):
                c1 = ppA.tile([128, 512], F32, tag="s", name="c1")
                c2 = ppA.tile([128, 256], F32, tag="s", name="c2")
                for j in range(HPC):
                    nc.tensor.matmul(
                        c1, lhsT=OT[j][0:DK, t * 128:(t + 1) * 128],
                        rhs=wosb[:, j, 0:512],
                        start=(j == 0), stop=(j == HPC - 1))
                for j in range(HPC):
                    nc.tensor.matmul(
                        c2, lhsT=OT[j][0:DK, t * 128:(t + 1) * 128],
                        rhs=wosb[:, j, 512:768],
                        start=(j == 0), stop=(j == HPC - 1))
                ot = opool.tile([128, D], F32, tag="o", name="ot")
                nc.vector.tensor_copy(out=ot[:, 0:512], in_=c1)
                nc.vector.tensor_copy(out=ot[:, 512:768], in_=c2)
                nc.sync.dma_start(out=out_d[t * 128:(t + 1) * 128, :], in_=ot)




def _emit_v3(nc, tc, xT_d, wqk_d, wv_d, bqk_d, bv_d, wo_d, out_d, dbg={}):
    """Software-pipelined per-head attention.

    PE cost on TRN2 is (output free size) x cycles/row regardless of
    contraction depth, so: AV matmuls contract the full 128-row k-block
    (half the matmuls of the old 64-split), V is projected directly into
    natural [seq, dim] layout (no PE transposes), and everything runs in
    bf16 (1.0 cycles/row). Phase A of head j+1 and phase C are emitted
    interleaved into phase B's exp-bound stretches so the PE fills the
    Activation engine's pacing gaps.
    """
    import contextlib
    ctx = contextlib.ExitStack()
    with ctx:
        wpool = ctx.enter_context(tc.tile_pool(name="wpool", bufs=1))
        persist = ctx.enter_context(tc.tile_pool(name="persist", bufs=1))
        hpool = ctx.enter_context(tc.tile_pool(name="hpool", bufs=2))
        xpool = ctx.enter_context(tc.tile_pool(name="xpool", bufs=8))
        epool = ctx.enter_context(tc.tile_pool(name="epool", bufs=4))
        rpool = ctx.enter_context(tc.tile_pool(name="rpool", bufs=2))
        opool = ctx.enter_context(tc.tile_pool(name="opool", bufs=2))
        # PSUM: ppS 2x2 banks (score groups) + ppO 2x1 (O accum) +
        # ppA 1x1 (QK proj / phase C c1) + ppT 1x1 (V proj / phase C c2)
        ppS = ctx.enter_context(tc.tile_pool(name="ppS", bufs=2, space="PSUM"))
        ppO = ctx.enter_context(tc.tile_pool(name="ppO", bufs=2, space="PSUM"))
        ppA = ctx.enter_context(tc.tile_pool(name="ppA", bufs=1, space="PSUM"))
        ppT = ctx.enter_context(tc.tile_pool(name="ppT", bufs=1, space="PSUM"))

        wqk = wpool.tile([128, HPC, 6, 128], BF16)
        nc.sync.dma_start(out=wqk, in_=wqk_d.rearrange("j c p m -> p j c m"))
        wv = wpool.tile([128, HPC, 6, DK], BF16)
        nc.sync.dma_start(out=wv, in_=wv_d.rearrange("j c p m -> p j c m"))
        bqk = wpool.tile([128, HPC], F32)
        nc.sync.dma_start(out=bqk, in_=bqk_d)
        bv = wpool.tile([1, HPC, DK], BF16)
        nc.sync.dma_start(out=bv, in_=bv_d)
        # wo packed for stacked phase C: heads 0,1 -> rows 0:128; head 2 alone
        wo = wpool.tile([DK, HPC, D], BF16)
        nc.sync.dma_start(out=wo, in_=wo_d.rearrange("j d m -> d j m"))
        wo01 = wpool.tile([128, D], BF16)
        nc.sync.dma_start(out=wo01[0:DK, :], in_=wo_d[0])
        nc.sync.dma_start(out=wo01[DK:128, :], in_=wo_d[1])
        ones = wpool.tile([1, 128], BF16)
        nc.vector.memset(ones, 1.0)
        # exp shift: scaled scores reach ~9 on these inputs; e4m3 tops at 448
        # (no inf), so exp(s/8 - 4) keeps 2x headroom. The shift cancels in
        # the normalization because the sums row rides the same es.
        bneg = wpool.tile([128, 1], F32)
        nc.vector.memset(bneg, -4.0)

        # OTP holds heads 0 (rows 0:64) and 1 (rows 64:128); OT2 head 2
        OTP = persist.tile([128, S], BF16, tag="otp", name="otp")
        OT2 = persist.tile([DK, S], BF16, tag="ot2", name="ot2")

        tiles = {}
        xqs = {}

        def alloc_head(j):
            QT = hpool.tile([128, S], BF16, tag="qt", name=f"qt{j}")
            KT = hpool.tile([128, S], BF16, tag="kt", name=f"kt{j}")
            V = hpool.tile([128, NKB // 2, 2, DK + 1], BF16, tag="v",
                           name=f"v{j}")
            nc.vector.memset(V[:, :, :, DK], 1.0)
            tiles[j] = (QT, KT, V)

        def emit_x_dma(ci):
            # x chunks are shared by all three heads; all 8 stay in SBUF
            xq = xpool.tile([128, 6, XCH], BF16, tag="x", name="xq")
            nc.sync.dma_start(
                out=xq,
                in_=xT_d.rearrange("(c p) q -> p c q", p=128)[
                    :, :, ci * XCH:(ci + 1) * XCH])
            xqs[ci] = xq

        def emit_A_chunk(j, ci):
            if ci == 0:
                alloc_head(j)
            QT, KT, V = tiles[j]
            cs = slice(ci * XCH, (ci + 1) * XCH)
            xq = xqs[ci]
            # Q|K packed projection
            ps = ppA.tile([128, XCH], F32, tag="pa", name="ps")
            for c in range(6):
                nc.tensor.matmul(ps, lhsT=wqk[:, j, c, :], rhs=xq[:, c, :],
                                 start=(c == 0), stop=(c == 5))
            nc.vector.tensor_scalar_add(
                out=QT[0:64, cs], in0=ps[0:64, :], scalar1=bqk[0:64, j:j + 1])
            nc.vector.tensor_scalar_add(
                out=KT[64:128, cs], in0=ps[64:128, :],
                scalar1=bqk[64:128, j:j + 1])
            # duplicate halves so S^T can alternate PE row tiles
            nc.sync.dma_start(out=QT[64:128, cs], in_=QT[0:64, cs])
            nc.sync.dma_start(out=KT[0:64, cs], in_=KT[64:128, cs])
            # V directly in natural [seq, dim] layout: x^T chunk as lhsT
            pv = ppT.tile([128, 2, 2, DK], F32, tag="pt", name="pv")
            for i in range(4):
                for c in range(6):
                    nc.tensor.matmul(
                        pv[:, i // 2, i % 2, :],
                        lhsT=xq[:, c, i * 128:(i + 1) * 128],
                        rhs=wv[:, j, c, :], start=(c == 0), stop=False)
                nc.tensor.matmul(pv[:, i // 2, i % 2, :], lhsT=ones,
                                 rhs=bv[:, j, :], start=False, stop=True)
            nc.vector.tensor_copy(
                out=V[:, ci * 2:(ci + 1) * 2, :, 0:DK], in_=pv)

        def emit_B_qi(j, qi, pre_group=None):
            QT, KT, V = tiles[j]
            qs = qi * QC
            poa = ppO.tile([DK + 1, QC], F32, tag="oa", name="poa")
            ng = NKB // 2

            def emit_av(g, es):
                # full 128-row k-block contraction per matmul
                nc.tensor.matmul(poa, lhsT=V[:, g, 0, :], rhs=es[:, 0, :],
                                 start=(g == 0), stop=False,
                                 skip_group_check=True)
                nc.tensor.matmul(poa, lhsT=V[:, g, 1, :], rhs=es[:, 1, :],
                                 start=False, stop=(g == ng - 1),
                                 skip_group_check=True)

            prev = None  # AV lags one group so PE never waits on exp
            for g in range(ng):
                if pre_group is not None:
                    pre_group(g)
                pss = ppS.tile([128, 2, QC], F32, tag="s", name="pss")
                h0 = (g % 2) * 64  # alternate PE row tiles across groups
                h1 = 64 - h0
                kb0, kb1 = 2 * g, 2 * g + 1
                nc.tensor.matmul(
                    pss[:, 0, :], lhsT=KT[h0:h0 + 64, kb0 * 128:kb0 * 128 + 128],
                    rhs=QT[h0:h0 + 64, qs:qs + QC], start=True, stop=True)
                nc.tensor.matmul(
                    pss[:, 1, :], lhsT=KT[h1:h1 + 64, kb1 * 128:kb1 * 128 + 128],
                    rhs=QT[h1:h1 + 64, qs:qs + QC], start=True, stop=True)
                es = epool.tile([128, 2, QC], BF16, tag="e", name="es")
                nc.scalar.activation(
                    out=es, in_=pss,
                    func=mybir.ActivationFunctionType.Exp, scale=0.125)
                if dbg and j == 2 and qi == 0 and g == 0:
                    nc.sync.dma_start(out=dbg["es"], in_=es)
                if prev is not None:
                    emit_av(*prev)
                prev = (g, es)
            emit_av(*prev)
            # normalize: recip of sums row, broadcast across partitions,
            # fused psum-evacuation multiply into bf16 OT
            ssum = rpool.tile([DK + 1, QC], F32, tag="sr", name="ssum")
            nc.vector.reciprocal(out=ssum[DK:DK + 1, :], in_=poa[DK:DK + 1, :])
            srow = rpool.tile([1, QC], F32, tag="s0", name="srow")
            nc.sync.dma_start(out=srow, in_=ssum[DK:DK + 1, :])
            rbc = rpool.tile([DK, QC], F32, tag="r", name="rbc")
            nc.gpsimd.partition_broadcast(rbc, srow, channels=DK)
            if dbg and j == 2 and qi == 0:
                nc.sync.dma_start(out=dbg["rbc"], in_=rbc)
            if j == 0:
                nc.vector.tensor_mul(out=OTP[0:DK, qs:qs + QC],
                                     in0=poa[0:DK, :], in1=rbc)
            elif j == 1:
                # engines can't shift partitions: stage then DMA into 64:128
                o1 = rpool.tile([DK, QC], BF16, tag="o1", name="o1")
                nc.vector.tensor_mul(out=o1, in0=poa[0:DK, :], in1=rbc)
                nc.sync.dma_start(out=OTP[DK:128, qs:qs + QC], in_=o1)
            else:
                nc.vector.tensor_mul(out=OT2[:, qs:qs + QC],
                                     in0=poa[0:DK, :], in1=rbc)

        def emit_C_qi(qi):
            for t in range(qi * QC // 128, (qi + 1) * QC // 128):
                ts = slice(t * 128, (t + 1) * 128)
                c1 = ppA.tile([128, 512], F32, tag="pa", name="c1")
                nc.tensor.matmul(c1, lhsT=OTP[:, ts], rhs=wo01[:, 0:512],
                                 start=True, stop=False)
                nc.tensor.matmul(c1, lhsT=OT2[:, ts], rhs=wo[:, 2, 0:512],
                                 start=False, stop=True)
                c2 = ppT.tile([128, 256], F32, tag="pt", name="c2")
                nc.tensor.matmul(c2, lhsT=OTP[:, ts], rhs=wo01[:, 512:768],
                                 start=True, stop=False)
                nc.tensor.matmul(c2, lhsT=OT2[:, ts], rhs=wo[:, 2, 512:768],
                                 start=False, stop=True)
                ot = opool.tile([128, D], F32, tag="o", name="ot")
                nc.vector.tensor_copy(out=ot[:, 0:512], in_=c1)
                nc.vector.tensor_copy(out=ot[:, 512:768], in_=c2)
                nc.sync.dma_start(out=out_d[ts, :], in_=ot)

        # head 0 phase A is pipelined into B(0, qi=0): chunk c is emitted
        # before score group 2(c-1), exactly one chunk ahead of the k-blocks
        # that need it, so the Act engine starts exp'ing almost immediately.
        for ci in range(S // XCH):
            emit_x_dma(ci)
        emit_A_chunk(0, 0)

        def a0_filler(g):
            c = g // 2 + 1
            if g % 2 == 0 and c < S // XCH:
                emit_A_chunk(0, c)

        for j in range(HPC):
            for qi in range(NQC):
                emit_B_qi(j, qi,
                          pre_group=a0_filler if j == 0 and qi == 0 else None)
                if j < HPC - 1:
                    emit_A_chunk(j + 1, qi)
                else:
                    emit_C_qi(qi)

        if dbg:
            QT2, KT2, V2 = tiles[2]
            nc.sync.dma_start(out=dbg["qt"], in_=QT2)
            nc.sync.dma_start(out=dbg["kt"], in_=KT2)
            nc.sync.dma_start(out=dbg["v"], in_=V2)
            nc.sync.dma_start(out=dbg["otp"], in_=OTP)
            nc.sync.dma_start(out=dbg["ot2"], in_=OT2)


def _emit_v2(nc, tc, xT_d, wp_d, bp_d, wo_d, out_d, exp_group=4):
    """Per-head pipeline; S^T psum in bf16 when exp_group=4 (2048-wide exp)."""
    import contextlib
    ctx = contextlib.ExitStack()
    with ctx:
        wpool = ctx.enter_context(tc.tile_pool(name="wpool", bufs=1))
        persist = ctx.enter_context(tc.tile_pool(name="persist", bufs=1))
        hpool = ctx.enter_context(tc.tile_pool(name="hpool", bufs=2))
        xpool = ctx.enter_context(tc.tile_pool(name="xpool", bufs=2))
        epool = ctx.enter_context(tc.tile_pool(name="epool", bufs=4))
        rpool = ctx.enter_context(tc.tile_pool(name="rpool", bufs=1))
        opool = ctx.enter_context(tc.tile_pool(name="opool", bufs=2))
        # one shared PSUM pool for S-groups/proj/transposes/phase C
        # (3 slots of 2 banks) + the two O accumulators (1 bank each)
        ppS = ctx.enter_context(tc.tile_pool(name="ppS", bufs=3, space="PSUM"))
        ppO = ctx.enter_context(tc.tile_pool(name="ppO", bufs=1, space="PSUM"))
        ppA = ppS

        SDT = BF16 if exp_group == 4 else F32
        NG = NKB // exp_group

        wsb = wpool.tile([128, HPC, 2, 6, 128], F32R)
        nc.sync.dma_start(out=wsb, in_=wp_d.rearrange("j g c p m -> p j g c m"))
        bsb = wpool.tile([128, HPC, 2], F32)
        nc.sync.dma_start(out=bsb, in_=bp_d)
        wosb = wpool.tile([DK, HPC, D], F32R)
        nc.sync.dma_start(out=wosb, in_=wo_d.rearrange("j d m -> d j m"))
        ident = wpool.tile([128, 128], BF16)
        make_identity(nc, ident)

        OT = [persist.tile([DK + 1, S], OT_DT, tag=f"ot{j}", name=f"ot{j}")
              for j in range(HPC)]

        def emit_c(cqi):
            for t in range(cqi * QC // 128, (cqi + 1) * QC // 128):
                c1 = ppO.tile([128, 512], F32, tag="oa", name="c1")
                c2 = ppO.tile([128, 256], F32, tag="ob", name="c2")
                for jj in range(HPC):
                    nc.tensor.matmul(
                        c1, lhsT=OT[jj][0:DK, t * 128:(t + 1) * 128],
                        rhs=wosb[:, jj, 0:512],
                        start=(jj == 0), stop=(jj == HPC - 1))
                for jj in range(HPC):
                    nc.tensor.matmul(
                        c2, lhsT=OT[jj][0:DK, t * 128:(t + 1) * 128],
                        rhs=wosb[:, jj, 512:768],
                        start=(jj == 0), stop=(jj == HPC - 1))
                ot = opool.tile([128, D], F32, tag="o", name="ot")
                nc.vector.tensor_copy(out=ot[:, 0:512], in_=c1)
                nc.vector.tensor_copy(out=ot[:, 512:768], in_=c2)
                nc.sync.dma_start(
                    out=out_d[t * 128:(t + 1) * 128, :], in_=ot)

        n_xch = S // XCH
        for j in range(HPC):
            # ---- phase A for head j ----
            QT = hpool.tile([128, S], BF16, tag="qt", name="qt")
            KT = hpool.tile([128, S], BF16, tag="kt", name="kt")
            VT = hpool.tile([128, S], BF16, tag="vt", name="vt")
            V = hpool.tile([128, NKB, DK + 1], BF16, tag="v", name="v")
            nc.vector.memset(V[:, :, DK], 1.0)
            for ci in range(n_xch):
                xq = xpool.tile([128, 6, XCH], F32R, tag="x", name="xq")
                nc.sync.dma_start(
                    out=xq,
                    in_=xT_d.rearrange("(c p) q -> p c q", p=128)[
                        :, :, ci * XCH:(ci + 1) * XCH])
                cs = slice(ci * XCH, (ci + 1) * XCH)
                # group 0: (Q | K)
                ps = ppA.tile([128, XCH], F32, tag="s", name="ps")
                for c in range(6):
                    nc.tensor.matmul(
                        ps, lhsT=wsb[:, j, 0, c, :], rhs=xq[:, c, :],
                        start=(c == 0), stop=(c == 5))
                nc.vector.tensor_scalar_add(
                    out=QT[0:64, cs], in0=ps[0:64, :],
                    scalar1=bsb[0:64, j, 0:1])
                nc.vector.tensor_scalar_add(
                    out=KT[64:128, cs], in0=ps[64:128, :],
                    scalar1=bsb[64:128, j, 0:1])
                # group 1: (V | V) duplicated
                ps2 = ppA.tile([128, XCH], F32, tag="s", name="ps2")
                for c in range(6):
                    nc.tensor.matmul(
                        ps2, lhsT=wsb[:, j, 1, c, :], rhs=xq[:, c, :],
                        start=(c == 0), stop=(c == 5))
                nc.vector.tensor_scalar_add(
                    out=VT[:, cs], in0=ps2, scalar1=bsb[:, j, 1:2])
                # V natural layout via PE transposes (chunk's k-blocks)
                for kb in range(ci * XCH // 128, (ci + 1) * XCH // 128):
                    pt = ppA.tile([128, 128], BF16, tag="s", name="pt")
                    nc.tensor.transpose(
                        pt, VT[:, kb * 128:(kb + 1) * 128], ident)
                    nc.vector.tensor_copy(
                        out=V[:, kb, 0:DK], in_=pt[:, 0:DK])
            # duplicate halves: Q lower->upper, K upper->lower
            nc.sync.dma_start(out=QT[64:128, :], in_=QT[0:64, :])
            nc.sync.dma_start(out=KT[0:64, :], in_=KT[64:128, :])

            # ---- phase B for head j ----
            for qi in range(NQC):
                qs = qi * QC
                poa = ppO.tile([DK + 1, QC], F32, tag="oa", name="poa")
                pob = ppO.tile([DK + 1, QC], F32, tag="ob", name="pob")
                for g in range(NG):
                    pss = ppS.tile([128, exp_group, QC], SDT, tag="s",
                                   name="pss")
                    # T0 row-tile: first half of the group's k-blocks;
                    # T8: second half (separate PSUM banks)
                    hg = exp_group // 2
                    for i in range(hg):
                        kb = g * exp_group + i
                        nc.tensor.matmul(
                            pss[:, i, :],
                            lhsT=KT[0:64, kb * 128:(kb + 1) * 128],
                            rhs=QT[0:64, qs:qs + QC],
                            start=True, stop=True)
                    for i in range(hg):
                        kb = g * exp_group + hg + i
                        nc.tensor.matmul(
                            pss[:, hg + i, :],
                            lhsT=KT[64:128, kb * 128:(kb + 1) * 128],
                            rhs=QT[64:128, qs:qs + QC],
                            start=True, stop=True)
                    es = epool.tile([128, exp_group, QC], BF16, tag="e",
                                    name="es")
                    nc.scalar.activation(
                        out=es, in_=pss,
                        func=mybir.ActivationFunctionType.Exp, scale=0.125)
                    for s in range(exp_group):
                        kb = g * exp_group + s
                        first = g == 0 and s == 0
                        last = g == NG - 1 and s == exp_group - 1
                        nc.tensor.matmul(
                            poa, lhsT=V[0:64, kb, :], rhs=es[0:64, s, :],
                            start=first, stop=last, skip_group_check=True)
                        nc.tensor.matmul(
                            pob, lhsT=V[64:128, kb, :], rhs=es[64:128, s, :],
                            start=first, stop=last, skip_group_check=True)
                nc.vector.tensor_copy(out=OT[j][:, qs:qs + QC], in_=poa)
                nc.vector.tensor_add(
                    out=OT[j][:, qs:qs + QC],
                    in0=OT[j][:, qs:qs + QC], in1=pob)
                nc.vector.reciprocal(
                    out=OT[j][DK:DK + 1, qs:qs + QC],
                    in_=OT[j][DK:DK + 1, qs:qs + QC])
                srow = rpool.tile([1, QC], OT_DT, tag="sr", name="srow")
                nc.sync.dma_start(
                    out=srow, in_=OT[j][DK:DK + 1, qs:qs + QC])
                rbc = rpool.tile([DK + 1, QC], OT_DT, tag="r", name="rbc")
                nc.gpsimd.partition_broadcast(rbc, srow, channels=DK + 1)
                nc.vector.tensor_mul(
                    out=OT[j][0:DK, qs:qs + QC],
                    in0=OT[j][0:DK, qs:qs + QC], in1=rbc[0:DK, :])

        # ---- phase C: output projection (borrows psumO slots) ----
        for cqi in range(NQC):
            emit_c(cqi)




# ---------------------------------------------------------------------------
# host side
# ---------------------------------------------------------------------------

KERNEL_MODE = "v3"


def shard_inputs(x, Wq, bq, Wk, bk, Wv, bv, Wo, bo, mode=None):
    """Build the 8 per-core input maps."""
    mode = mode or KERNEL_MODE
    if mode == "v3":
        return shard_inputs_v3(x, Wq, bq, Wk, bk, Wv, bv, Wo, bo)
    if mode.startswith("v2"):
        return shard_inputs_v2(x, Wq, bq, Wk, bk, Wv, bv, Wo, bo)
    return shard_inputs_v1(x, Wq, bq, Wk, bk, Wv, bv, Wo, bo)


def shard_inputs_v3(x, Wq, bq, Wk, bk, Wv, bv, Wo, bo):
    bf16 = mybir.dt.np(BF16)
    x = np.asarray(x, np.float32)
    Wq, Wk, Wv = (np.asarray(a, np.float32) for a in (Wq, Wk, Wv))
    bq, bk, bv = (np.asarray(a, np.float32) for a in (bq, bk, bv))
    Wo = np.asarray(Wo, np.float32)
    in_maps = []
    for c in range(N_CORES):
        b, g = divmod(c, 4)
        heads = [3 * g + j for j in range(HPC)]
        wqk = np.empty((HPC, 6, 128, 128), np.float32)
        wv_ = np.empty((HPC, 6, 128, DK), np.float32)
        bqk = np.zeros((128, HPC), np.float32)
        bv_ = np.zeros((1, HPC, DK), np.float32)
        wo_ = np.empty((HPC, DK, D), np.float32)
        for j, h in enumerate(heads):
            sl = slice(64 * h, 64 * h + 64)
            wqk[j, :, :, 0:64] = Wq[sl].T.reshape(6, 128, 64)
            wqk[j, :, :, 64:128] = Wk[sl].T.reshape(6, 128, 64)
            wv_[j] = Wv[sl].T.reshape(6, 128, 64)
            bqk[0:64, j] = bq[sl]
            bqk[64:128, j] = bk[sl]
            bv_[0, j] = bv[sl]
            wo_[j] = Wo[:, sl].T
        in_maps.append({
            "xT": np.ascontiguousarray(x[b].T).astype(bf16),
            "wqk": wqk.astype(bf16), "wv": wv_.astype(bf16),
            "bqk": bqk, "bv": bv_.astype(bf16), "wo": wo_.astype(bf16),
        })
    return in_maps


def shard_inputs_v2(x, Wq, bq, Wk, bk, Wv, bv, Wo, bo):
    x = np.asarray(x, np.float32)
    Wq, Wk, Wv = (np.asarray(a, np.float32) for a in (Wq, Wk, Wv))
    bq, bk, bv = (np.asarray(a, np.float32) for a in (bq, bk, bv))
    Wo = np.asarray(Wo, np.float32)
    in_maps = []
    for c in range(N_CORES):
        b, g = divmod(c, 4)
        heads = [3 * g + j for j in range(HPC)]
        wp = np.empty((HPC, 2, 6, 128, 128), np.float32)
        bp = np.zeros((128, HPC, 2), np.float32)
        wo = np.empty((HPC, DK, D), np.float32)
        for j, h in enumerate(heads):
            sl = slice(64 * h, 64 * h + 64)
            wp[j, 0, :, :, 0:64] = Wq[sl].T.reshape(6, 128, 64)
            wp[j, 0, :, :, 64:128] = Wk[sl].T.reshape(6, 128, 64)
            wp[j, 1, :, :, 0:64] = Wv[sl].T.reshape(6, 128, 64)
            wp[j, 1, :, :, 64:128] = Wv[sl].T.reshape(6, 128, 64)
            bp[0:64, j, 0] = bq[sl]
            bp[64:128, j, 0] = bk[sl]
            bp[0:64, j, 1] = bv[sl]
            bp[64:128, j, 1] = bv[sl]
            wo[j] = Wo[:, sl].T
        in_maps.append({
            "xT": np.ascontiguousarray(x[b].T),
            "wp": wp, "bp": bp, "wo": wo,
        })
    return in_maps


def shard_inputs_v1(x, Wq, bq, Wk, bk, Wv, bv, Wo, bo):
    """Build the 8 per-core input maps."""
    x = np.asarray(x, np.float32)
    Ws = {0: np.asarray(Wq, np.float32), 1: np.asarray(Wk, np.float32),
          2: np.asarray(Wv, np.float32)}
    bs = {0: np.asarray(bq, np.float32), 1: np.asarray(bk, np.float32),
          2: np.asarray(bv, np.float32)}
    Wo = np.asarray(Wo, np.float32)
    in_maps = []
    for c in range(N_CORES):
        b, g = divmod(c, 4)
        heads = [3 * g + j for j in range(HPC)]
        wp = np.empty((5, 6, 128, 128), np.float32)
        bp = np.zeros((128, 5), np.float32)
        for gi, (mA, mB) in enumerate(PROJ_GROUPS):
            for half, (j, kind) in ((0, mA), (1, mB)):
                h = heads[j]
                Wh = Ws[kind][64 * h:64 * h + 64, :]       # [64, 768]
                chunks = Wh.T.reshape(6, 128, 64)          # [c, p, 64]
                wp[gi, :, :, half * 64:half * 64 + 64] = chunks
                bp[half * 64:half * 64 + 64, gi] = bs[kind][64 * h:64 * h + 64]
        wo = np.empty((HPC, DK, D), np.float32)
        for j in range(HPC):
            h = heads[j]
            wo[j] = Wo[:, 64 * h:64 * h + 64].T
        in_maps.append({
            "xT": np.ascontiguousarray(x[b].T),
            "wp": wp, "bp": bp, "wo": wo,
        })
    return in_maps


def assemble_output(parts, bo):
    out = np.empty((B, S, D), np.float32)
    for b in range(B):
        acc = parts[4 * b]["out"].astype(np.float32).copy()
        for c in range(4 * b + 1, 4 * b + 4):
            acc += parts[c]["out"]
        out[b] = acc + np.asarray(bo, np.float32)[None, :]
    return out


_RUNNER = None


def _make_runner(nc):
    """Reusable PJRT runner (mirrors bass2jax.run_bass_via_pjrt multi-core)."""
    import jax
    import jax.numpy as jnp
    from jax.experimental.shard_map import shard_map
    from jax.sharding import Mesh, PartitionSpec
    from concourse import bass2jax

    bass2jax.install_neuronx_cc_hook()

    partition_name = (nc.partition_id_tensor.name
                      if nc.partition_id_tensor else None)
    in_names, out_names, out_avals = [], [], []
    for alloc in nc.m.functions[0].allocations:
        if not isinstance(alloc, mybir.MemoryLocationSet):
            continue
        name = alloc.memorylocations[0].name
        if alloc.kind == "ExternalInput":
            if name != partition_name:
                in_names.append(name)
        elif alloc.kind == "ExternalOutput":
            out_names.append(name)
            out_avals.append(jax.core.ShapedArray(
                tuple(alloc.tensor_shape), mybir.dt.np(alloc.dtype)))
    n_params = len(in_names)
    n_outs = len(out_names)
    all_in_names = list(in_names) + list(out_names)
    if partition_name is not None:
        all_in_names.append(partition_name)
    donate = tuple(range(n_params, n_params + n_outs))

    def _body(*args):
        operands = list(args)
        if partition_name is not None:
            operands.append(bass2jax.partition_id_tensor())
        outs = bass2jax._bass_exec_p.bind(
            *operands,
            out_avals=tuple(out_avals),
            in_names=tuple(all_in_names),
            out_names=tuple(out_names),
            lowering_input_output_aliases=(),
            sim_require_finite=True,
            sim_require_nnan=True,
            nc=nc,
        )
        return tuple(outs)

    devices = jax.devices()[:N_CORES]
    mesh = Mesh(np.asarray(devices), ("core",))
    in_specs = (PartitionSpec("core"),) * (n_params + n_outs)
    out_specs = (PartitionSpec("core"),) * n_outs
    sharded = jax.jit(
        shard_map(_body, mesh=mesh, in_specs=in_specs, out_specs=out_specs,
                  check_rep=False),
        donate_argnums=donate, keep_unused=True)

    def run(in_maps):
        per_core = [[np.asarray(m[name]) for name in in_names]
                    for m in in_maps]
        concat_in = [np.concatenate([per_core[c][i] for c in range(N_CORES)],
                                    axis=0) for i in range(n_params)]
        zeros = [np.zeros((N_CORES * av.shape[0], *av.shape[1:]), av.dtype)
                 for av in out_avals]
        outs = sharded(*concat_in, *zeros)
        return [
            {name: np.asarray(outs[i]).reshape(N_CORES, *out_avals[i].shape)[c]
             for i, name in enumerate(out_names)}
            for c in range(N_CORES)
        ]

    run.sharded = sharded
    run.in_names = in_names
    run.out_names = out_names
    run.out_avals = out_avals
    run.n_params = n_params
    return run


def get_runner():
    global _RUNNER
    if _RUNNER is None:
        nc = build_program()
        _RUNNER = _make_runner(nc)
    return _RUNNER


def kernel(x, Wq, bq, Wk, bk, Wv, bv, Wo, bo):
    run = get_runner()
    in_maps = shard_inputs(x, Wq, bq, Wk, bk, Wv, bv, Wo, bo)
    parts = run(in_maps)
    return assemble_output(parts, bo)

